# revision 12
# baseline (speedup 1.0000x reference)
# kernel.py — fused causal ReLU-attention (qkv proj + q@k^T + relu/causal + @v)
# for Trainium2, 8 NeuronCores, batch-parallel (1 batch element per core).
#
# Self-contained: hardcodes shapes B,T,C = 8,1024,768, nh=12, hs=64.
import os
import sys

for p in ("/opt/trn_rl_repo", "/root/.axon_site", "/root/.axon_site/_ro/trn_rl_repo"):
    if os.path.isdir(p) and p not in sys.path:
        sys.path.append(p)

import numpy as np

import concourse.bass as bass
import concourse.mybir as mybir
import concourse.tile as tile
from concourse import bacc
from concourse import bass_utils
from concourse.masks import make_identity

F32 = mybir.dt.float32
F32R = mybir.dt.float32r
AF = mybir.ActivationFunctionType
ALU = mybir.AluOpType

B, T, C = 8, 1024, 768
NH, HS = 12, 64
SCALE = 1.0 / 8.0  # 1/sqrt(64)
P = 128
NT = T // P    # 8 t-tiles
KC = C // P    # 6 c-tiles (contraction)
NPAIR = NH // 2  # 6 head pairs
TCH = 512      # t1 chunk size
NCH = T // TCH  # 2 chunks


def r32(ap):
    return ap.bitcast(F32R)


def build_nc(n_cores=8):
    nc = bacc.Bacc("TRN2", target_bir_lowering=False, debug=False,
                   num_devices=n_cores)

    x_d = nc.dram_tensor("x", [T, C], F32, kind="ExternalInput").ap()
    # f32r: PE full-rate fp32 mode; np view is float32 either way
    w_d = nc.dram_tensor("w", [C, 3 * C], F32R, kind="ExternalInput").ap()
    b_d = nc.dram_tensor("b", [3 * C], F32, kind="ExternalInput").ap()
    y_d = nc.dram_tensor("y", [T, C], F32, kind="ExternalOutput").ap()

    with tile.TileContext(nc) as tc:
        _emit(nc, tc, x_d, w_d, b_d, y_d)

    nc.compile()
    return nc


def _emit(nc, tc, x_d, w_d, b_d, y_d):
    from contextlib import ExitStack

    with ExitStack() as ctx:
        pp = ctx.enter_context(tc.tile_pool(name="persist", bufs=1))
        tr_ps = ctx.enter_context(
            tc.tile_pool(name="tr_psum", bufs=2, space="PSUM"))

        # ---- constants ----
        ident = pp.tile([P, P], F32, tag="ident", name="ident")
        make_identity(nc, ident[:])

        # qk bias: cols 0..1536 rearranged [(m p) -> p m] -> [128, 12]
        bqk = pp.tile([P, 12], F32, tag="bqk", name="bqk")
        nc.sync.dma_start(bqk[:], b_d[0:2 * C].rearrange("(a p) -> p a", p=P))

        # v bias broadcast to all partitions: [128, 768]
        bv_row = pp.tile([1, C], F32, tag="bvrow", name="bvrow")
        nc.sync.dma_start(bv_row[:], b_d[2 * C:3 * C].rearrange("(o a) -> o a", o=1))
        bv = pp.tile([P, C], F32, tag="bv", name="bv")
        nc.gpsimd.partition_broadcast(bv[:], bv_row[0:1, :])

        # diagonal relu mask, pre-scaled: cols 0..127 = (col>=part ? SCALE : 0),
        # cols 128.. = SCALE
        smask = pp.tile([P, TCH], F32, tag="smask", name="smask")
        nc.gpsimd.memset(smask[:], SCALE)
        nc.gpsimd.affine_select(
            out=smask[:, 0:P], in_=smask[:, 0:P],
            compare_op=ALU.is_ge, fill=0.0, base=0,
            pattern=[[1, P]], channel_multiplier=-1)

        # ---- persistent activations ----
        qkT = [pp.tile([P, T], F32R, tag=f"qkT{m}", name=f"qkT{m}") for m in range(12)]
        v_sb = [pp.tile([P, C], F32R, tag=f"v{i}", name=f"v{i}") for i in range(NT)]
        y_sb = [pp.tile([P, C], F32, tag=f"y{i}", name=f"y{i}") for i in range(NT)]

        # ================= phase A-C: x load/transpose + qkv =================
        with ExitStack() as c1:
            wp = c1.enter_context(tc.tile_pool(name="wpool", bufs=1))
            xs = c1.enter_context(tc.tile_pool(name="xstream", bufs=3))
            xtp = c1.enter_context(tc.tile_pool(name="xT", bufs=1))
            qps = c1.enter_context(
                tc.tile_pool(name="qkv_psum", bufs=2, space="PSUM"))

            w_sb = [wp.tile([P, 3 * C], F32R, tag=f"w{k}", name=f"w{k}") for k in range(KC)]
            for k in range(KC):
                nc.sync.dma_start(w_sb[k][:], w_d[P * k:P * (k + 1), :])

            xT = [xtp.tile([P, T], F32R, tag=f"xT{k}", name=f"xT{k}") for k in range(KC)]

            # load x tile-by-tile, transpose on PE
            for i in range(NT):
                xt = xs.tile([P, C], F32, tag="xs", name="xs")
                nc.sync.dma_start(xt[:], x_d[P * i:P * (i + 1), :])
                for k in range(KC):
                    pt = tr_ps.tile([P, P], F32, tag="trp", name="trp")
                    nc.tensor.transpose(pt[:], xt[:, P * k:P * (k + 1)],
                                        ident[:])
                    if (i + k) % 2 == 0:
                        nc.scalar.activation(xT[k][:, P * i:P * (i + 1)],
                                             pt[:], AF.Copy)
                    else:
                        nc.vector.tensor_copy(xT[k][:, P * i:P * (i + 1)],
                                              pt[:])

            # ---- qk part: out qkT[m][:, tchunk] = W[:,m-tile].T @ xT ----
            for m in range(12):
                for t in range(NCH):
                    ps = qps.tile([P, TCH], F32, tag="qkvps", name="qkvps")
                    for k in range(KC):
                        nc.tensor.matmul(
                            ps[:],
                            w_sb[k][:, P * m:P * (m + 1)],
                            xT[k][:, TCH * t:TCH * (t + 1)],
                            start=(k == 0), stop=(k == KC - 1))
                    # evict with bias add (partition = qkv col)
                    if m % 2 == 0:
                        nc.scalar.activation(qkT[m][:, TCH * t:TCH * (t + 1)],
                                             ps[:], AF.Identity,
                                             bias=bqk[:, m:m + 1])
                    else:
                        nc.vector.tensor_scalar(
                            qkT[m][:, TCH * t:TCH * (t + 1)], ps[:],
                            bqk[:, m:m + 1], None, ALU.add)

            # ---- v part: out v_sb[i][:, nchunk] = xT[i-tile].T @ W[:,vcols] ----
            for i in range(NT):
                for nck, (n0, n1) in enumerate(((0, 512), (512, 768))):
                    ps = qps.tile([P, TCH], F32, tag="qkvps", name="qkvps")
                    for k in range(KC):
                        nc.tensor.matmul(
                            ps[:, 0:n1 - n0],
                            xT[k][:, P * i:P * (i + 1)],
                            w_sb[k][:, 2 * C + n0:2 * C + n1],
                            start=(k == 0), stop=(k == KC - 1))
                    nc.vector.tensor_tensor(
                        v_sb[i][:, n0:n1], ps[:, 0:n1 - n0],
                        bv[:, n0:n1], ALU.add)

        # ================= phase D-E: attention per head pair ================
        with ExitStack() as c2:
            ap_pool = c2.enter_context(tc.tile_pool(name="attp", bufs=10))
            yt_pool = c2.enter_context(tc.tile_pool(name="yT", bufs=4))
            a_ps = c2.enter_context(
                tc.tile_pool(name="att_psum", bufs=2, space="PSUM"))
            y_ps = c2.enter_context(
                tc.tile_pool(name="y_psum", bufs=4, space="PSUM"))

            for j in range(NPAIR):
                qt, kt = qkT[j], qkT[6 + j]
                # per-head yT at partitions 0..63 (f32r matmul cannot target
                # dst partition base 64 — walrus ISA check)
                yTh = [yt_pool.tile([64, T], F32, tag="yT", name="yT")
                       for _ in range(2)]
                for c in range(NCH):
                    c_lo, c_hi = TCH * c, TCH * (c + 1)
                    rmax = c_hi // P  # t2 tiles 0..rmax-1
                    pieces = []  # (r, hh, off, n, sbuf_tile)

                    # ---- att matmuls: attT[t2, t1] = k^T q (per head) ----
                    for r in range(rmax):
                        t2_0 = P * r
                        off = max(0, t2_0 - c_lo)  # col offset within chunk
                        n = TCH - off              # valid cols
                        diag = t2_0 >= c_lo
                        for hh in range(2):
                            h0 = 64 * hh
                            ps = a_ps.tile([P, TCH], F32, tag="aps", name="aps")
                            nc.tensor.matmul(
                                ps[:, 0:n],
                                kt[h0:h0 + 64, t2_0:t2_0 + P],
                                qt[h0:h0 + 64, c_lo + off:c_hi],
                                start=True, stop=True,
                                tile_position=(h0, 0))
                            at = ap_pool.tile([P, TCH], F32R, tag="attp", name="attp")
                            if diag:
                                # relu + causal mask + scale in one DVE op
                                nc.vector.scalar_tensor_tensor(
                                    at[:, 0:n], ps[:, 0:n], 0.0,
                                    smask[:, 0:n], ALU.max, ALU.mult)
                            else:
                                nc.scalar.activation(at[:, 0:n], ps[:, 0:n],
                                                     AF.Relu, scale=SCALE)
                            pieces.append((r, hh, off, n, at))

                    # ---- av matmuls: yT_h[d, t1] += att @ v accumulation ----
                    # one PSUM bank per head: a start=True zero-region is
                    # bank-wide, so the two heads must not share a bank
                    yp = [y_ps.tile([64, TCH], F32, tag="yps", name="yps")
                          for _ in range(2)]
                    for (r, hh, off, n, at) in pieces:
                        h0 = 64 * hh
                        nc.tensor.matmul(
                            yp[hh][:, off:off + n],
                            v_sb[r][:, P * j + h0:P * j + h0 + 64],
                            at[:, 0:n],
                            start=(r == 0), stop=(r == rmax - 1))
                    for hh in range(2):
                        if (c + hh) % 2 == 0:
                            nc.scalar.activation(yTh[hh][:, c_lo:c_hi],
                                                 yp[hh][:], AF.Copy)
                        else:
                            nc.vector.tensor_copy(yTh[hh][:, c_lo:c_hi],
                                                  yp[hh][:])

                # ---- transpose yT_h [d, t] -> y [t, d] blocks ----
                for i in range(NT):
                    for hh in range(2):
                        pt = tr_ps.tile([P, P], F32, tag="trp", name="trp")
                        nc.tensor.transpose(pt[:, 0:64],
                                            yTh[hh][:, P * i:P * (i + 1)],
                                            ident[0:64, 0:64])
                        c0 = P * j + 64 * hh
                        if (i + j + hh) % 2 == 0:
                            nc.vector.tensor_copy(y_sb[i][:, c0:c0 + 64],
                                                  pt[:, 0:64])
                        else:
                            nc.scalar.activation(y_sb[i][:, c0:c0 + 64],
                                                 pt[:, 0:64], AF.Copy)

            for i in range(NT):
                nc.sync.dma_start(y_d[P * i:P * (i + 1), :], y_sb[i][:])


def _ensure_ntff_hook():
    """Register the axon NTFF profiling hook if the image's antenv lacks
    axon_hooks (bass_utils hard-imports it on the trace=True path)."""
    import types
    try:
        from antenv import axon_hooks  # noqa: F401
        return
    except ImportError:
        pass
    import antenv
    mod = types.ModuleType("antenv.axon_hooks")
    mod._hook = None

    def set_axon_ntff_profile_hook(h):
        mod._hook = h

    def get_axon_ntff_profile_hook():
        return mod._hook

    mod.set_axon_ntff_profile_hook = set_axon_ntff_profile_hook
    mod.get_axon_ntff_profile_hook = get_axon_ntff_profile_hook
    sys.modules["antenv.axon_hooks"] = mod
    antenv.axon_hooks = mod
    try:
        from trn_agent_boot.trn_boot import _ntff_profile_via_ctypes
        hook = _ntff_profile_via_ctypes("/opt/axon/libaxon_pjrt.so")
        if hook is not None:
            mod._hook = hook
    except Exception:
        pass


_NC_CACHE = None


def _get_nc():
    global _NC_CACHE
    if _NC_CACHE is None:
        _NC_CACHE = build_nc()
    return _NC_CACHE


def kernel(x, W_attn, b_attn, _trace=False):
    x = np.ascontiguousarray(np.asarray(x, dtype=np.float32))
    w = np.ascontiguousarray(np.asarray(W_attn, dtype=np.float32))
    b = np.ascontiguousarray(np.asarray(b_attn, dtype=np.float32))
    assert x.shape == (B, T, C) and w.shape == (C, 3 * C) and b.shape == (3 * C,)

    if _trace:
        _ensure_ntff_hook()
    nc = _get_nc()
    in_maps = [{"x": x[i], "w": w, "b": b} for i in range(B)]
    res = bass_utils.run_bass_kernel_spmd(
        nc, in_maps, core_ids=list(range(B)), trace=_trace)
    y = np.stack([res.results[i]["y"] for i in range(B)], axis=0)
    if _trace:
        kernel.last_result = res
    return y


# revision 14
# speedup vs baseline: 1.0354x; 1.0354x over previous
# kernel.py — fused causal ReLU-attention (qkv proj + q@k^T + relu/causal + @v)
# for Trainium2, 8 NeuronCores, batch-parallel (1 batch element per core).
#
# Self-contained: hardcodes shapes B,T,C = 8,1024,768, nh=12, hs=64.
import os
import sys

for p in ("/opt/trn_rl_repo", "/root/.axon_site", "/root/.axon_site/_ro/trn_rl_repo"):
    if os.path.isdir(p) and p not in sys.path:
        sys.path.append(p)

import numpy as np

import concourse.bass as bass
import concourse.mybir as mybir
import concourse.tile as tile
from concourse import bacc
from concourse import bass_utils
from concourse.masks import make_identity

F32 = mybir.dt.float32
F32R = mybir.dt.float32r
AF = mybir.ActivationFunctionType
ALU = mybir.AluOpType

B, T, C = 8, 1024, 768
NH, HS = 12, 64
SCALE = 1.0 / 8.0  # 1/sqrt(64)
P = 128
NT = T // P    # 8 t-tiles
KC = C // P    # 6 c-tiles (contraction)
NPAIR = NH // 2  # 6 head pairs
TCH = 512      # t1 chunk size
NCH = T // TCH  # 2 chunks


def r32(ap):
    return ap.bitcast(F32R)


def build_nc(n_cores=8):
    nc = bacc.Bacc("TRN2", target_bir_lowering=False, debug=False,
                   num_devices=n_cores)

    x_d = nc.dram_tensor("x", [T, C], F32, kind="ExternalInput").ap()
    # f32r: PE full-rate fp32 mode; np view is float32 either way
    w_d = nc.dram_tensor("w", [C, 3 * C], F32R, kind="ExternalInput").ap()
    b_d = nc.dram_tensor("b", [3 * C], F32, kind="ExternalInput").ap()
    y_d = nc.dram_tensor("y", [T, C], F32, kind="ExternalOutput").ap()

    with tile.TileContext(nc) as tc:
        _emit(nc, tc, x_d, w_d, b_d, y_d)

    nc.compile()
    return nc


def _emit(nc, tc, x_d, w_d, b_d, y_d):
    from contextlib import ExitStack

    with ExitStack() as ctx:
        pp = ctx.enter_context(tc.tile_pool(name="persist", bufs=1))

        # ---- constants ----
        ident = pp.tile([P, P], F32, tag="ident", name="ident")
        make_identity(nc, ident[:])

        # qk bias: cols 0..1536 rearranged [(m p) -> p m] -> [128, 12]
        bqk = pp.tile([P, 12], F32, tag="bqk", name="bqk")
        nc.sync.dma_start(bqk[:], b_d[0:2 * C].rearrange("(a p) -> p a", p=P))

        # v bias broadcast to all partitions: [128, 768]
        bv_row = pp.tile([1, C], F32, tag="bvrow", name="bvrow")
        nc.sync.dma_start(bv_row[:], b_d[2 * C:3 * C].rearrange("(o a) -> o a", o=1))
        bv = pp.tile([P, C], F32, tag="bv", name="bv")
        nc.gpsimd.partition_broadcast(bv[:], bv_row[0:1, :])

        # master relu/causal mask, pre-scaled by SCALE:
        #   cols [0,384) = 0 ; [384,512) = (col-384>=part ? SCALE : 0) ;
        #   [512,896) = SCALE
        # slice M[:, 384-z : 384-z+N] masks a piece whose zero-prefix is z
        # and whose causal-diagonal 128-block sits at piece cols [z, z+128)
        mstr = pp.tile([P, 896], F32, tag="mstr", name="mstr")
        nc.gpsimd.memset(mstr[:, 0:384], 0.0)
        nc.gpsimd.memset(mstr[:, 384:896], SCALE)
        nc.gpsimd.affine_select(
            out=mstr[:, 384:512], in_=mstr[:, 384:512],
            compare_op=ALU.is_ge, fill=0.0, base=0,
            pattern=[[1, P]], channel_multiplier=-1)

        # ---- persistent activations ----
        qkT = [pp.tile([P, T], F32R, tag=f"qkT{m}", name=f"qkT{m}") for m in range(12)]
        v_sb = [pp.tile([P, C], F32R, tag=f"v{i}", name=f"v{i}") for i in range(NT)]
        y_sb = [pp.tile([P, C], F32, tag=f"y{i}", name=f"y{i}") for i in range(NT)]

        # ================= phase A-C: x load/transpose + qkv =================
        with ExitStack() as c1:
            wp = c1.enter_context(tc.tile_pool(name="wpool", bufs=1))
            xs = c1.enter_context(tc.tile_pool(name="xstream", bufs=4))
            xtp = c1.enter_context(tc.tile_pool(name="xT", bufs=1))
            qps = c1.enter_context(
                tc.tile_pool(name="qkv_psum", bufs=2, space="PSUM"))
            trA = c1.enter_context(
                tc.tile_pool(name="trA_psum", bufs=2, space="PSUM"))

            # x DMAs first: the x transposes gate the first qkv matmuls
            x_tiles = []
            for i in range(NT):
                xt = xs.tile([P, C], F32, tag="xs", name="xs")
                nc.sync.dma_start(xt[:], x_d[P * i:P * (i + 1), :])
                x_tiles.append(xt)

            w_sb = [wp.tile([P, 3 * C], F32R, tag=f"w{k}", name=f"w{k}")
                    for k in range(KC)]
            for k in range(KC):
                nc.sync.dma_start(w_sb[k][:], w_d[P * k:P * (k + 1), :])

            xT = [xtp.tile([P, T], F32R, tag=f"xT{k}", name=f"xT{k}")
                  for k in range(KC)]

            for i in range(NT):
                xt = x_tiles[i]
                for k in range(KC):
                    pt = trA.tile([P, P], F32, tag="trp", name="trp")
                    nc.tensor.transpose(pt[:], xt[:, P * k:P * (k + 1)],
                                        ident[:])
                    if (i + k) % 2 == 0:
                        nc.scalar.activation(xT[k][:, P * i:P * (i + 1)],
                                             pt[:], AF.Copy)
                    else:
                        nc.vector.tensor_copy(xT[k][:, P * i:P * (i + 1)],
                                              pt[:])

            # ---- qk part: out qkT[m][:, tchunk] = W[:,m-tile].T @ xT ----
            for m in range(12):
                for t in range(NCH):
                    ps = qps.tile([P, TCH], F32, tag="qkvps", name="qkvps")
                    for k in range(KC):
                        nc.tensor.matmul(
                            ps[:],
                            w_sb[k][:, P * m:P * (m + 1)],
                            xT[k][:, TCH * t:TCH * (t + 1)],
                            start=(k == 0), stop=(k == KC - 1))
                    # evict with bias add (partition = qkv col)
                    if m % 2 == 0:
                        nc.scalar.activation(qkT[m][:, TCH * t:TCH * (t + 1)],
                                             ps[:], AF.Identity,
                                             bias=bqk[:, m:m + 1])
                    else:
                        nc.vector.tensor_scalar(
                            qkT[m][:, TCH * t:TCH * (t + 1)], ps[:],
                            bqk[:, m:m + 1], None, ALU.add)

            # ---- v part: v_sb[i][:, nchunk] = xT[i-tile].T @ W[:,vcols] ----
            for i in range(NT):
                for (n0, n1) in ((0, 512), (512, 768)):
                    ps = qps.tile([P, TCH], F32, tag="qkvps", name="qkvps")
                    for k in range(KC):
                        nc.tensor.matmul(
                            ps[:, 0:n1 - n0],
                            xT[k][:, P * i:P * (i + 1)],
                            w_sb[k][:, 2 * C + n0:2 * C + n1],
                            start=(k == 0), stop=(k == KC - 1))
                    nc.vector.tensor_tensor(
                        v_sb[i][:, n0:n1], ps[:, 0:n1 - n0],
                        bv[:, n0:n1], ALU.add)

        # ================= phase D-E: attention per head pair ================
        with ExitStack() as c2:
            ap_pool = c2.enter_context(tc.tile_pool(name="attp", bufs=10))
            yt_pool = c2.enter_context(tc.tile_pool(name="yT", bufs=4))
            a_ps = c2.enter_context(
                tc.tile_pool(name="att_psum", bufs=3, space="PSUM"))
            y_ps = c2.enter_context(
                tc.tile_pool(name="y_psum", bufs=3, space="PSUM"))
            trE = c2.enter_context(
                tc.tile_pool(name="trE_psum", bufs=2, space="PSUM"))

            for j in range(NPAIR):
                qt, kt = qkT[j], qkT[6 + j]
                # per-head yT at partitions 0..63 (f32r matmul cannot target
                # dst partition base 64 — walrus ISA check)
                yTh = [yt_pool.tile([64, T], F32, tag="yT", name="yT")
                       for _ in range(2)]
                for c in range(NCH):
                    c_lo, c_hi = TCH * c, TCH * (c + 1)
                    rmax = c_hi // P
                    pieces = []  # (r, hh, offp, n, sbuf_tile)

                    # ---- att: attT[t2, t1] = k^T q ; min moving dim 256 ----
                    for r in range(rmax):
                        t2_0 = P * r
                        off = max(0, t2_0 - c_lo)
                        offp = min(off, TCH - 256)   # widen tails to N>=256
                        z = off - offp               # zero-prefix inside piece
                        n = TCH - offp
                        diag = t2_0 >= c_lo
                        for hh in range(2):
                            h0 = 64 * hh
                            ps = a_ps.tile([P, TCH], F32, tag="aps",
                                           name="aps")
                            nc.tensor.matmul(
                                ps[:, 0:n],
                                kt[h0:h0 + 64, t2_0:t2_0 + P],
                                qt[h0:h0 + 64, c_lo + offp:c_hi],
                                start=True, stop=True,
                                tile_position=(h0, 0))
                            at = ap_pool.tile([P, TCH], F32R, tag="attp",
                                              name="attp")
                            if diag:
                                # relu + zero-prefix + causal + scale, one op
                                nc.vector.scalar_tensor_tensor(
                                    at[:, 0:n], ps[:, 0:n], 0.0,
                                    mstr[:, 384 - z:384 - z + n],
                                    ALU.max, ALU.mult)
                            else:
                                nc.scalar.activation(at[:, 0:n], ps[:, 0:n],
                                                     AF.Relu, scale=SCALE)
                            pieces.append((r, hh, offp, n, at))

                    # ---- av: yT_h[d, t1] += v.T-laid accumulation ----
                    yp = [y_ps.tile([64, TCH], F32, tag="yps", name="yps")
                          for _ in range(2)]
                    for (r, hh, offp, n, at) in pieces:
                        h0 = 64 * hh
                        nc.tensor.matmul(
                            yp[hh][:, offp:offp + n],
                            v_sb[r][:, P * j + h0:P * j + h0 + 64],
                            at[:, 0:n],
                            start=(r == 0), stop=(r == rmax - 1))
                    for hh in range(2):
                        if (c + hh) % 2 == 0:
                            nc.scalar.activation(yTh[hh][:, c_lo:c_hi],
                                                 yp[hh][:], AF.Copy)
                        else:
                            nc.vector.tensor_copy(yTh[hh][:, c_lo:c_hi],
                                                  yp[hh][:])

                # ---- transpose yT_h [d, t] -> y [t, d] blocks ----
                for i in range(NT):
                    for hh in range(2):
                        pt = trE.tile([P, P], F32, tag="trE", name="trE")
                        nc.tensor.transpose(pt[:, 0:64],
                                            yTh[hh][:, P * i:P * (i + 1)],
                                            ident[0:64, 0:64])
                        c0 = P * j + 64 * hh
                        if (i + hh) % 2 == 0:
                            nc.vector.tensor_copy(y_sb[i][:, c0:c0 + 64],
                                                  pt[:, 0:64])
                        else:
                            nc.scalar.activation(y_sb[i][:, c0:c0 + 64],
                                                 pt[:, 0:64], AF.Copy)

            for i in range(NT):
                nc.sync.dma_start(y_d[P * i:P * (i + 1), :], y_sb[i][:])


def _ensure_ntff_hook():
    """Register the axon NTFF profiling hook if the image's antenv lacks
    axon_hooks (bass_utils hard-imports it on the trace=True path)."""
    import types
    try:
        from antenv import axon_hooks  # noqa: F401
        return
    except ImportError:
        pass
    import antenv
    mod = types.ModuleType("antenv.axon_hooks")
    mod._hook = None

    def set_axon_ntff_profile_hook(h):
        mod._hook = h

    def get_axon_ntff_profile_hook():
        return mod._hook

    mod.set_axon_ntff_profile_hook = set_axon_ntff_profile_hook
    mod.get_axon_ntff_profile_hook = get_axon_ntff_profile_hook
    sys.modules["antenv.axon_hooks"] = mod
    antenv.axon_hooks = mod
    try:
        from trn_agent_boot.trn_boot import _ntff_profile_via_ctypes
        hook = _ntff_profile_via_ctypes("/opt/axon/libaxon_pjrt.so")
        if hook is not None:
            mod._hook = hook
    except Exception:
        pass


_NC_CACHE = None


def _get_nc():
    global _NC_CACHE
    if _NC_CACHE is None:
        _NC_CACHE = build_nc()
    return _NC_CACHE


def kernel(x, W_attn, b_attn, _trace=False):
    x = np.ascontiguousarray(np.asarray(x, dtype=np.float32))
    w = np.ascontiguousarray(np.asarray(W_attn, dtype=np.float32))
    b = np.ascontiguousarray(np.asarray(b_attn, dtype=np.float32))
    assert x.shape == (B, T, C) and w.shape == (C, 3 * C) and b.shape == (3 * C,)

    if _trace:
        _ensure_ntff_hook()
    nc = _get_nc()
    in_maps = [{"x": x[i], "w": w, "b": b} for i in range(B)]
    res = bass_utils.run_bass_kernel_spmd(
        nc, in_maps, core_ids=list(range(B)), trace=_trace)
    y = np.stack([res.results[i]["y"] for i in range(B)], axis=0)
    if _trace:
        kernel.last_result = res
    return y


# revision 15
# speedup vs baseline: 1.0570x; 1.0208x over previous
# kernel.py — fused causal ReLU-attention (qkv proj + q@k^T + relu/causal + @v)
# for Trainium2, 8 NeuronCores, batch-parallel (1 batch element per core).
#
# Self-contained: hardcodes shapes B,T,C = 8,1024,768, nh=12, hs=64.
import os
import sys

for p in ("/opt/trn_rl_repo", "/root/.axon_site", "/root/.axon_site/_ro/trn_rl_repo"):
    if os.path.isdir(p) and p not in sys.path:
        sys.path.append(p)

import numpy as np

import concourse.bass as bass
import concourse.mybir as mybir
import concourse.tile as tile
from concourse import bacc
from concourse import bass_utils
from concourse.masks import make_identity

F32 = mybir.dt.float32
F32R = mybir.dt.float32r
AF = mybir.ActivationFunctionType
ALU = mybir.AluOpType

B, T, C = 8, 1024, 768
NH, HS = 12, 64
SCALE = 1.0 / 8.0  # 1/sqrt(64)
P = 128
NT = T // P    # 8 t-tiles
KC = C // P    # 6 c-tiles (contraction)
NPAIR = NH // 2  # 6 head pairs
TCH = 512      # t1 chunk size
NCH = T // TCH  # 2 chunks


def r32(ap):
    return ap.bitcast(F32R)


def build_nc(n_cores=8):
    nc = bacc.Bacc("TRN2", target_bir_lowering=False, debug=False,
                   num_devices=n_cores)

    x_d = nc.dram_tensor("x", [T, C], F32, kind="ExternalInput").ap()
    # f32r: PE full-rate fp32 mode; np view is float32 either way
    w_d = nc.dram_tensor("w", [C, 3 * C], F32R, kind="ExternalInput").ap()
    b_d = nc.dram_tensor("b", [3 * C], F32, kind="ExternalInput").ap()
    y_d = nc.dram_tensor("y", [T, C], F32, kind="ExternalOutput").ap()

    with tile.TileContext(nc) as tc:
        _emit(nc, tc, x_d, w_d, b_d, y_d)

    nc.compile()
    return nc


def _emit(nc, tc, x_d, w_d, b_d, y_d):
    from contextlib import ExitStack

    with ExitStack() as ctx:
        pp = ctx.enter_context(tc.tile_pool(name="persist", bufs=1))

        # ---- constants ----
        ident = pp.tile([P, P], F32, tag="ident", name="ident")
        make_identity(nc, ident[:])

        # qk bias: cols 0..1536 rearranged [(m p) -> p m] -> [128, 12]
        bqk = pp.tile([P, 12], F32, tag="bqk", name="bqk")
        nc.sync.dma_start(bqk[:], b_d[0:2 * C].rearrange("(a p) -> p a", p=P))

        # v bias broadcast to all partitions: [128, 768]
        bv_row = pp.tile([1, C], F32, tag="bvrow", name="bvrow")
        nc.sync.dma_start(bv_row[:], b_d[2 * C:3 * C].rearrange("(o a) -> o a", o=1))
        bv = pp.tile([P, C], F32, tag="bv", name="bv")
        nc.gpsimd.partition_broadcast(bv[:], bv_row[0:1, :])

        # master relu/causal mask, pre-scaled by SCALE:
        #   cols [0,384) = 0 ; [384,512) = (col-384>=part ? SCALE : 0) ;
        #   [512,896) = SCALE
        # slice M[:, 384-z : 384-z+N] masks a piece whose zero-prefix is z
        # and whose causal-diagonal 128-block sits at piece cols [z, z+128)
        mstr = pp.tile([P, 896], F32, tag="mstr", name="mstr")
        nc.gpsimd.memset(mstr[:, 0:384], 0.0)
        nc.gpsimd.memset(mstr[:, 384:896], SCALE)
        nc.gpsimd.affine_select(
            out=mstr[:, 384:512], in_=mstr[:, 384:512],
            compare_op=ALU.is_ge, fill=0.0, base=0,
            pattern=[[1, P]], channel_multiplier=-1)

        # ---- persistent activations ----
        qkT = [pp.tile([P, T], F32R, tag=f"qkT{m}", name=f"qkT{m}") for m in range(12)]
        v_sb = [pp.tile([P, C], F32R, tag=f"v{i}", name=f"v{i}") for i in range(NT)]
        y_sb = [pp.tile([P, C], F32, tag=f"y{i}", name=f"y{i}") for i in range(NT)]

        # ================= phase A-C: x load/transpose + qkv =================
        with ExitStack() as c1:
            wp = c1.enter_context(tc.tile_pool(name="wpool", bufs=1))
            xs = c1.enter_context(tc.tile_pool(name="xstream", bufs=4))
            xtp = c1.enter_context(tc.tile_pool(name="xT", bufs=1))
            qps = c1.enter_context(
                tc.tile_pool(name="qkv_psum", bufs=2, space="PSUM"))
            trA = c1.enter_context(
                tc.tile_pool(name="trA_psum", bufs=2, space="PSUM"))

            # x DMAs first: the x transposes gate the first qkv matmuls
            x_tiles = []
            for i in range(NT):
                xt = xs.tile([P, C], F32, tag="xs", name="xs")
                nc.sync.dma_start(xt[:], x_d[P * i:P * (i + 1), :])
                x_tiles.append(xt)

            w_sb = [wp.tile([P, 3 * C], F32R, tag=f"w{k}", name=f"w{k}")
                    for k in range(KC)]
            for k in range(KC):
                nc.sync.dma_start(w_sb[k][:, 0:2 * C],
                                  w_d[P * k:P * (k + 1), 0:2 * C])
            for k in range(KC):
                nc.sync.dma_start(w_sb[k][:, 2 * C:3 * C],
                                  w_d[P * k:P * (k + 1), 2 * C:3 * C])

            xT = [xtp.tile([P, T], F32R, tag=f"xT{k}", name=f"xT{k}")
                  for k in range(KC)]

            for i in range(NT):
                xt = x_tiles[i]
                for k in range(KC):
                    pt = trA.tile([P, P], F32, tag="trp", name="trp")
                    nc.tensor.transpose(pt[:], xt[:, P * k:P * (k + 1)],
                                        ident[:])
                    if (i + k) % 2 == 0:
                        nc.scalar.activation(xT[k][:, P * i:P * (i + 1)],
                                             pt[:], AF.Copy)
                    else:
                        nc.vector.tensor_copy(xT[k][:, P * i:P * (i + 1)],
                                              pt[:])

            # ---- qk part: out qkT[m][:, tchunk] = W[:,m-tile].T @ xT ----
            for m in range(12):
                for t in range(NCH):
                    ps = qps.tile([P, TCH], F32, tag="qkvps", name="qkvps")
                    for k in range(KC):
                        nc.tensor.matmul(
                            ps[:],
                            w_sb[k][:, P * m:P * (m + 1)],
                            xT[k][:, TCH * t:TCH * (t + 1)],
                            start=(k == 0), stop=(k == KC - 1))
                    # evict with bias add (partition = qkv col)
                    if m % 2 == 0:
                        nc.scalar.activation(qkT[m][:, TCH * t:TCH * (t + 1)],
                                             ps[:], AF.Identity,
                                             bias=bqk[:, m:m + 1])
                    else:
                        nc.vector.tensor_scalar(
                            qkT[m][:, TCH * t:TCH * (t + 1)], ps[:],
                            bqk[:, m:m + 1], None, ALU.add)

            # ---- v part: v_sb[i][:, nchunk] = xT[i-tile].T @ W[:,vcols] ----
            for i in range(NT):
                for (n0, n1) in ((0, 512), (512, 768)):
                    ps = qps.tile([P, TCH], F32, tag="qkvps", name="qkvps")
                    for k in range(KC):
                        nc.tensor.matmul(
                            ps[:, 0:n1 - n0],
                            xT[k][:, P * i:P * (i + 1)],
                            w_sb[k][:, 2 * C + n0:2 * C + n1],
                            start=(k == 0), stop=(k == KC - 1))
                    nc.vector.tensor_tensor(
                        v_sb[i][:, n0:n1], ps[:, 0:n1 - n0],
                        bv[:, n0:n1], ALU.add)

        # ================= phase D-E: attention per head pair ================
        with ExitStack() as c2:
            ap_pool = c2.enter_context(tc.tile_pool(name="attp", bufs=26))
            yt_pool = c2.enter_context(tc.tile_pool(name="yT", bufs=4))
            a_ps = c2.enter_context(
                tc.tile_pool(name="att_psum", bufs=3, space="PSUM"))
            y_ps = c2.enter_context(
                tc.tile_pool(name="y_psum", bufs=3, space="PSUM"))
            trE = c2.enter_context(
                tc.tile_pool(name="trE_psum", bufs=2, space="PSUM"))

            for j in range(NPAIR):
                qt, kt = qkT[j], qkT[6 + j]
                # per-head yT at partitions 0..63 (f32r matmul cannot target
                # dst partition base 64 — walrus ISA check)
                yTh = [yt_pool.tile([64, T], F32, tag="yT", name="yT")
                       for _ in range(2)]
                # ---- att for BOTH chunks first: keeps the PE stream
                # dense while DVE/ACT evictions trail behind (HAM stays warm)
                chunk_pieces = []
                for c in range(NCH):
                    c_lo, c_hi = TCH * c, TCH * (c + 1)
                    rmax = c_hi // P
                    pieces = []  # (r, hh, offp, n, sbuf_tile)
                    for r in range(rmax):
                        t2_0 = P * r
                        off = max(0, t2_0 - c_lo)
                        offp = min(off, TCH - 256)   # widen tails to N>=256
                        z = off - offp               # zero-prefix inside piece
                        n = TCH - offp
                        diag = t2_0 >= c_lo
                        for hh in range(2):
                            h0 = 64 * hh
                            ps = a_ps.tile([P, TCH], F32, tag="aps",
                                           name="aps")
                            nc.tensor.matmul(
                                ps[:, 0:n],
                                kt[h0:h0 + 64, t2_0:t2_0 + P],
                                qt[h0:h0 + 64, c_lo + offp:c_hi],
                                start=True, stop=True,
                                tile_position=(h0, 0))
                            at = ap_pool.tile([P, TCH], F32R, tag="attp",
                                              name="attp")
                            if diag:
                                # relu + zero-prefix + causal + scale, one op
                                nc.vector.scalar_tensor_tensor(
                                    at[:, 0:n], ps[:, 0:n], 0.0,
                                    mstr[:, 384 - z:384 - z + n],
                                    ALU.max, ALU.mult)
                            else:
                                nc.scalar.activation(at[:, 0:n], ps[:, 0:n],
                                                     AF.Relu, scale=SCALE)
                            pieces.append((r, hh, offp, n, at))
                    chunk_pieces.append(pieces)

                # ---- av: yT_h[d, t1] += att @ v accumulation ----
                for c in range(NCH):
                    c_lo, c_hi = TCH * c, TCH * (c + 1)
                    rmax = c_hi // P
                    yp = [y_ps.tile([64, TCH], F32, tag="yps", name="yps")
                          for _ in range(2)]
                    for (r, hh, offp, n, at) in chunk_pieces[c]:
                        h0 = 64 * hh
                        nc.tensor.matmul(
                            yp[hh][:, offp:offp + n],
                            v_sb[r][:, P * j + h0:P * j + h0 + 64],
                            at[:, 0:n],
                            start=(r == 0), stop=(r == rmax - 1))
                    for hh in range(2):
                        if (c + hh) % 2 == 0:
                            nc.scalar.activation(yTh[hh][:, c_lo:c_hi],
                                                 yp[hh][:], AF.Copy)
                        else:
                            nc.vector.tensor_copy(yTh[hh][:, c_lo:c_hi],
                                                  yp[hh][:])

                # ---- transpose yT_h [d, t] -> y [t, d] blocks ----
                for i in range(NT):
                    for hh in range(2):
                        pt = trE.tile([P, P], F32, tag="trE", name="trE")
                        nc.tensor.transpose(pt[:, 0:64],
                                            yTh[hh][:, P * i:P * (i + 1)],
                                            ident[0:64, 0:64])
                        c0 = P * j + 64 * hh
                        if (i + hh) % 2 == 0:
                            nc.vector.tensor_copy(y_sb[i][:, c0:c0 + 64],
                                                  pt[:, 0:64])
                        else:
                            nc.scalar.activation(y_sb[i][:, c0:c0 + 64],
                                                 pt[:, 0:64], AF.Copy)

            for i in range(NT):
                nc.sync.dma_start(y_d[P * i:P * (i + 1), :], y_sb[i][:])


def _ensure_ntff_hook():
    """Register the axon NTFF profiling hook if the image's antenv lacks
    axon_hooks (bass_utils hard-imports it on the trace=True path)."""
    import types
    try:
        from antenv import axon_hooks  # noqa: F401
        return
    except ImportError:
        pass
    import antenv
    mod = types.ModuleType("antenv.axon_hooks")
    mod._hook = None

    def set_axon_ntff_profile_hook(h):
        mod._hook = h

    def get_axon_ntff_profile_hook():
        return mod._hook

    mod.set_axon_ntff_profile_hook = set_axon_ntff_profile_hook
    mod.get_axon_ntff_profile_hook = get_axon_ntff_profile_hook
    sys.modules["antenv.axon_hooks"] = mod
    antenv.axon_hooks = mod
    try:
        from trn_agent_boot.trn_boot import _ntff_profile_via_ctypes
        hook = _ntff_profile_via_ctypes("/opt/axon/libaxon_pjrt.so")
        if hook is not None:
            mod._hook = hook
    except Exception:
        pass


_NC_CACHE = None


def _get_nc():
    global _NC_CACHE
    if _NC_CACHE is None:
        _NC_CACHE = build_nc()
    return _NC_CACHE


def kernel(x, W_attn, b_attn, _trace=False):
    x = np.ascontiguousarray(np.asarray(x, dtype=np.float32))
    w = np.ascontiguousarray(np.asarray(W_attn, dtype=np.float32))
    b = np.ascontiguousarray(np.asarray(b_attn, dtype=np.float32))
    assert x.shape == (B, T, C) and w.shape == (C, 3 * C) and b.shape == (3 * C,)

    if _trace:
        _ensure_ntff_hook()
    nc = _get_nc()
    in_maps = [{"x": x[i], "w": w, "b": b} for i in range(B)]
    res = bass_utils.run_bass_kernel_spmd(
        nc, in_maps, core_ids=list(range(B)), trace=_trace)
    y = np.stack([res.results[i]["y"] for i in range(B)], axis=0)
    if _trace:
        kernel.last_result = res
    return y
